# revision 1
# baseline (speedup 1.0000x reference)
"""Trainium2 Bass kernel for nn_CombinedLoss_54631984005443.

Computes, over inputs pc1_0 (8,1024,3), pc1_1 (8,512,3), pc1_3 (8,1024,1),
pc2 (8,1024,3), pc3 (8,1024,3):

  loss = conf_mse + 0.5*chamfer(pc1_0, pc2) + 0.5*sum_b sinkhorn_emd(C_b)
         + chamfer(pc1_1, pc2)

Sharding: core b handles batch sample b.
 - The EMD (dominant cost: 100 Sinkhorn iterations on a 1024x1024 cost
   matrix) is per-batch -> perfectly data parallel (1 sample per core).
 - The cross-batch flattened chamfers are sharded by query rows: each core
   computes, for its 1024/512 "row" points, the min distance against ALL
   8192/4096 opposite points, in BOTH orientations, so every reduction is a
   local free-axis min. Cores emit small partial sums; the host just adds.

Key implementation choices:
 - d^2 matrices via a K=30 bf16 "triple-split" matmul: each coordinate x is
   split into 3 bf16 terms (x ~ h+m+l); 8 dominant cross products per
   coordinate plus 3-split norm features give d^2 exact to ~1e-6 absolute at
   full bf16 PE rate (fp32 matmul would be 4x slower).
 - Sinkhorn in multiplicative form with K~ = exp(-C/eps + ln(n)):
       u = 1/(K~ v);  v = 1/(K~^T u)
   which is algebraically identical to the reference's log-domain iteration
   (mu folded into K~), starting from v=1.
 - The two matvecs per iteration run on the TensorEngine with the vector as
   the stationary operand ([128,1] weight loads are ~free) in float32r
   (full-rate), accumulating over 8 contraction chunks into a [1,1024] PSUM
   row; DVE reciprocal -> tiny K=1 matmuls redistribute [1,1024] back to a
   [128,8] column layout for the next direction's stationary operand.
"""

import numpy as np
import ml_dtypes
from contextlib import ExitStack

import concourse.bass as bass
from concourse import bacc
import concourse.tile as tile
from concourse import mybir
from concourse.bass_utils import run_bass_kernel_spmd

B, N, NSEED = 8, 1024, 512
NITERS = 100
KF = 30  # feature rows for the d^2 matmul trick
F32 = mybir.dt.float32
F32R = mybir.dt.float32r
BF16 = mybir.dt.bfloat16
AF = mybir.ActivationFunctionType
ALU = mybir.AluOpType
AX = mybir.AxisListType
LOG_N = float(np.log(N))

_BF = ml_dtypes.bfloat16


def _split3(x):
    """float64 array -> three bf16 arrays h,m,l with h+m+l ~ x (to ~2^-27)."""
    h = x.astype(_BF)
    r = x - h.astype(np.float64)
    m = r.astype(_BF)
    l = (r - m.astype(np.float64)).astype(_BF)
    return h, m, l


def _features(pts):
    """pts [n,3] float -> (FU [30,n], FV [30,n]) bf16 feature matrices.

    FU(a) . FV(b) = |a|^2 + |b|^2 - 2 a.b = ||a-b||^2  (to ~1e-6 abs).
    """
    p = pts.astype(np.float64)
    n = p.shape[0]
    fu = np.zeros((KF, n), np.float64)
    fv = np.zeros((KF, n), np.float64)
    row = 0
    for c in range(3):
        h, m, l = (t.astype(np.float64) for t in _split3(p[:, c]))
        # product pairs covering (h+m+l)*(h+m+l) except l*l
        uparts = [h, h, m, m, h, l, m, l]
        vparts = [h, m, h, m, l, h, l, m]
        for uu, vv in zip(uparts, vparts):
            fu[row] = -2.0 * uu
            fv[row] = vv
            row += 1
    na = np.sum(p * p, axis=1)
    nh, nm, nl = (t.astype(np.float64) for t in _split3(na))
    for t in (nh, nm, nl):
        fu[row] = t
        fv[row] = 1.0
        row += 1
    for t in (nh, nm, nl):
        fu[row] = 1.0
        fv[row] = t
        row += 1
    assert row == KF
    return fu.astype(_BF), fv.astype(_BF)


def _P(nc, name, shape, dtype=BF16):
    return nc.declare_dram_parameter(name, list(shape), dtype, isOutput=False)


def build_program(niters=NITERS, with_a2=True, with_b=True, a2_parts=(0,1,2,3,'conf')):
    nc = bacc.Bacc("TRN2")

    fu10 = _P(nc, "fu10", [KF, N])          # FU(pc1_0 batch)
    fv10b = _P(nc, "fv10b", [KF, N])        # FV(pc1_0 batch)
    fu2 = _P(nc, "fu2", [KF, N])            # FU(pc2 batch)
    fv2b = _P(nc, "fv2b", [KF, N])          # FV(pc2 batch)
    fv10a = _P(nc, "fv10a", [KF, B * N])    # FV(pc1_0 all)
    fv2a = _P(nc, "fv2a", [KF, B * N])      # FV(pc2 all)
    fu11 = _P(nc, "fu11", [KF, NSEED])      # FU(pc1_1 batch)
    fv11a = _P(nc, "fv11a", [KF, B * NSEED])  # FV(pc1_1 all)
    fu3 = _P(nc, "fu3", [KF, N])            # FU(pc3 batch)
    pc13 = _P(nc, "pc13", [128, N // 128], F32)
    out = nc.declare_dram_parameter("out", [1, 8], F32, isOutput=True)

    NT = N // 128  # 8 row tiles per 1024 points

    with tile.TileContext(nc) as tc, ExitStack() as top:
        sfeat = top.enter_context(tc.tile_pool(name="sfeat", bufs=1))
        kmat = top.enter_context(tc.tile_pool(name="kmat", bufs=1))
        work = top.enter_context(tc.tile_pool(name="work", bufs=1))
        consts = top.enter_context(tc.tile_pool(name="consts", bufs=1))

        # ---- constants
        ones_col = consts.tile([128, 1], F32, tag="ones_col")
        nc.vector.memset(ones_col, 1.0)
        one11 = consts.tile([1, 1], F32, tag="one11")
        nc.vector.memset(one11, 1.0)
        lnN_vec = consts.tile([128, 1], F32, tag="lnN_vec")
        nc.vector.memset(lnN_vec, LOG_N)
        ones_row = consts.tile([1, 128], F32, tag="ones_row")
        nc.vector.memset(ones_row, 1.0)

        # ---- small (per-batch) feature tiles
        t_fu10 = sfeat.tile([KF, N], BF16, tag="fu10")
        t_fv10b = sfeat.tile([KF, N], BF16, tag="fv10b")
        t_fu2 = sfeat.tile([KF, N], BF16, tag="fu2")
        t_fv2b = sfeat.tile([KF, N], BF16, tag="fv2b")
        t_fu3 = sfeat.tile([KF, N], BF16, tag="fu3")
        t_pc13 = sfeat.tile([128, NT], F32, tag="pc13")
        for t, d in ((t_fu10, fu10), (t_fv10b, fv10b), (t_fu2, fu2),
                     (t_fv2b, fv2b), (t_fu3, fu3), (t_pc13, pc13)):
            nc.sync.dma_start(out=t, in_=d[:, :])

        # ---- persistent K matrices (fp32): K~ in both layouts + K~*C
        kn = [kmat.tile([128, N], F32R, tag=f"kn{c}", name=f"kn{c}") for c in range(NT)]
        kt = [kmat.tile([128, N], F32R, tag=f"kt{c}", name=f"kt{c}") for c in range(NT)]
        kc = [kmat.tile([128, N], F32R, tag=f"kc{c}", name=f"kc{c}") for c in range(NT)]

        # ---- misc work tiles
        out_sb = work.tile([1, 8], F32, tag="out_sb")
        nc.vector.memset(out_sb, 0.0)
        u_sb = work.tile([1, N], F32, tag="u_sb")
        v_sb = work.tile([1, N], F32, tag="v_sb")
        u_par = work.tile([128, NT], F32R, tag="u_par")
        v_par = work.tile([128, NT], F32R, tag="v_par")
        scale_vec = work.tile([128, 1], F32, tag="scale_vec")  # -1/eps

        # =================================================================
        # Phase A1: EMD cost matrices C (both orientations), eps, K~, K~*C
        # =================================================================
        with tc.tile_pool(name="cmat", bufs=1) as cmat, \
             tc.tile_pool(name="psA1", bufs=4, space="PSUM") as psA1, \
             tc.tile_pool(name="wA1", bufs=2) as wA1:
            cn = [cmat.tile([128, N], F32, tag=f"cn{c}", name=f"cn{c}") for c in range(NT)]
            ct = [cmat.tile([128, N], F32, tag=f"ct{c}", name=f"ct{c}") for c in range(NT)]
            eps_acc = wA1.tile([128, 2 * NT], F32, tag="eps_acc")

            for c in range(NT):  # row tile (i for cn, j for ct)
                for h in range(2):  # 512-wide column halves
                    sl = slice(h * 512, (h + 1) * 512)
                    ps = psA1.tile([128, 512], F32, tag="d2")
                    nc.tensor.matmul(ps, t_fu10[:, c * 128:(c + 1) * 128],
                                     t_fv2b[:, sl], start=True, stop=True)
                    nc.vector.tensor_scalar_max(out=ps, in0=ps, scalar1=0.0)
                    # C = sqrt(d2); per-partition running sums for eps
                    nc.scalar.activation(out=cn[c][:, sl], in_=ps, func=AF.Sqrt,
                                         accum_out=eps_acc[:, 2 * c + h:2 * c + h + 1])
                    ps2 = psA1.tile([128, 512], F32, tag="d2")
                    nc.tensor.matmul(ps2, t_fu2[:, c * 128:(c + 1) * 128],
                                     t_fv10b[:, sl], start=True, stop=True)
                    nc.vector.tensor_scalar_max(out=ps2, in0=ps2, scalar1=0.0)
                    nc.scalar.activation(out=ct[c][:, sl], in_=ps2, func=AF.Sqrt)

            # eps = 0.02 * mean(C);  scale_vec = -1/eps broadcast to [128,1]
            s_col = wA1.tile([128, 1], F32, tag="s_col")
            nc.vector.reduce_sum(out=s_col, in_=eps_acc, axis=AX.X)
            ps_s = psA1.tile([1, 1], F32, tag="sc", bufs=1)
            nc.tensor.matmul(ps_s, s_col, ones_col, start=True, stop=True)
            s_inv = wA1.tile([1, 1], F32, tag="s_inv")
            nc.vector.reciprocal(out=s_inv, in_=ps_s)  # 1/sum(C)
            # -1/eps = -(N*N)/(0.02*sum) = s_inv * (-N*N/0.02)
            nc.vector.tensor_scalar_mul(out=s_inv, in0=s_inv,
                                        scalar1=-float(N) * float(N) / 0.02)
            ps_b = psA1.tile([128, 1], F32, tag="scb", bufs=1)
            nc.tensor.matmul(ps_b, ones_row, s_inv, start=True, stop=True)
            nc.vector.tensor_copy(out=scale_vec, in_=ps_b)

            # K~ = exp(-C/eps + ln(N));  K~C = K~ * C  (KN layout)
            for c in range(NT):
                nc.scalar.activation(out=kn[c], in_=cn[c], func=AF.Exp,
                                     bias=lnN_vec, scale=scale_vec)
                nc.scalar.activation(out=kt[c], in_=ct[c], func=AF.Exp,
                                     bias=lnN_vec, scale=scale_vec)
                nc.vector.tensor_mul(out=kc[c], in0=kn[c], in1=cn[c])

        # =================================================================
        # Phase A2: chamfer partial sums + confidence partial
        # =================================================================
        if not with_a2:
            pass
        else:
         with tc.tile_pool(name="bfeat", bufs=1) as bfeat, \
             tc.tile_pool(name="psA2", bufs=4, space="PSUM") as psA2, \
             tc.tile_pool(name="wA2", bufs=2) as wA2:
            t_fv10a = bfeat.tile([KF, B * N], BF16, tag="fv10a")
            t_fv2a = bfeat.tile([KF, B * N], BF16, tag="fv2a")
            t_fu11 = bfeat.tile([KF, NSEED], BF16, tag="fu11")
            t_fv11a = bfeat.tile([KF, B * NSEED], BF16, tag="fv11a")
            for t0 in range(0, B * N, N):
                nc.sync.dma_start(out=t_fv10a[:, t0:t0 + N], in_=fv10a[:, t0:t0 + N])
                nc.sync.dma_start(out=t_fv2a[:, t0:t0 + N], in_=fv2a[:, t0:t0 + N])
            for t0 in range(0, B * NSEED, N):
                nc.sync.dma_start(out=t_fv11a[:, t0:t0 + N], in_=fv11a[:, t0:t0 + N])
            nc.sync.dma_start(out=t_fu11, in_=fu11[:, :])

            def nn_sum_quantity(qslot, t_fu, rows, t_fv, cols):
                """min over cols of d^2 per row point -> sum(sqrt) -> out_sb[0,qslot]."""
                ntile = rows // 128
                nn = cols // 512
                dmin = wA2.tile([128, ntile], F32, tag=f"dmin{qslot}")
                for t in range(ntile):
                    mins = wA2.tile([128, nn], F32, tag="mins")
                    for n in range(nn):
                        ps = psA2.tile([128, 512], F32, tag="chd2")
                        nc.tensor.matmul(ps, t_fu[:, t * 128:(t + 1) * 128],
                                         t_fv[:, n * 512:(n + 1) * 512],
                                         start=True, stop=True)
                        nc.vector.tensor_reduce(out=mins[:, n:n + 1], in_=ps,
                                                axis=AX.X, op=ALU.min)
                    nc.vector.tensor_reduce(out=dmin[:, t:t + 1], in_=mins,
                                            axis=AX.X, op=ALU.min)
                nc.vector.tensor_scalar_max(out=dmin, in0=dmin, scalar1=0.0)
                nc.scalar.activation(out=dmin, in_=dmin, func=AF.Sqrt)
                dsum = wA2.tile([128, 1], F32, tag="dsum")
                nc.vector.reduce_sum(out=dsum, in_=dmin, axis=AX.X)
                ps_q = psA2.tile([1, 1], F32, tag="q", bufs=1)
                nc.tensor.matmul(ps_q, dsum, ones_col, start=True, stop=True)
                nc.vector.tensor_copy(out=out_sb[:, qslot:qslot + 1], in_=ps_q)

            if 0 in a2_parts:
                nn_sum_quantity(0, t_fu10, N, t_fv2a, B * N)    # chamfer1 dist2 shard
            if 1 in a2_parts:
                nn_sum_quantity(1, t_fu2, N, t_fv10a, B * N)    # chamfer1 dist1 shard
            if 2 in a2_parts:
                nn_sum_quantity(2, t_fu11, NSEED, t_fv2a, B * N)  # chamfer2 dist2 shard
            if 3 in a2_parts:
                nn_sum_quantity(3, t_fu2, N, t_fv11a, B * NSEED)  # chamfer2 dist1 shard

            if 'conf' in a2_parts:
                # confidence: gt = exp(-min_j d(pc3_i, pc2_j)); sse vs pc1_3
                if 'conf' not in a2_parts:
                    m3 = None
                m3 = wA2.tile([128, 2 * NT], F32, tag="m3")
                for t in range(NT):
                    for h in range(2):
                        ps = psA2.tile([128, 512], F32, tag="chd2")
                        nc.tensor.matmul(ps, t_fu3[:, t * 128:(t + 1) * 128],
                                         t_fv2b[:, h * 512:(h + 1) * 512],
                                         start=True, stop=True)
                        nc.vector.tensor_reduce(out=m3[:, h * NT + t:h * NT + t + 1],
                                                in_=ps, axis=AX.X, op=ALU.min)
                gt = wA2.tile([128, NT], F32, tag="gt")
                nc.vector.tensor_tensor(out=gt, in0=m3[:, 0:NT], in1=m3[:, NT:2 * NT], op=ALU.min)
                nc.vector.tensor_scalar_max(out=gt, in0=gt, scalar1=0.0)
                nc.scalar.activation(out=gt, in_=gt, func=AF.Sqrt)
                nc.scalar.activation(out=gt, in_=gt, func=AF.Exp, scale=-1.0)
                diff = wA2.tile([128, NT], F32, tag="diff")
                nc.vector.tensor_sub(out=diff, in0=t_pc13, in1=gt)
                sse_junk = wA2.tile([128, NT], F32, tag="sse_junk")
                sse_col = wA2.tile([128, 1], F32, tag="sse_col")
                nc.vector.tensor_mul(out=sse_junk, in0=diff, in1=diff)
                nc.vector.reduce_sum(out=sse_col, in_=sse_junk, axis=AX.X)
                ps_q = psA2.tile([1, 1], F32, tag="q", bufs=1)
                nc.tensor.matmul(ps_q, sse_col, ones_col, start=True, stop=True)
                nc.vector.tensor_copy(out=out_sb[:, 4:5], in_=ps_q)

        # =================================================================
        # Phase B: Sinkhorn iterations + transport cost
        # =================================================================
        if not with_b:
            pass
        else:
         with tc.tile_pool(name="psB", bufs=1, space="PSUM") as psB, \
             tc.tile_pool(name="psBt", bufs=1, space="PSUM") as psBt:
            nc.vector.memset(v_par.bitcast(F32), 1.0)

            def half_iter(k_tiles, vec_par, vec_sb, par_out):
                """vec_sb = 1/(K vec_par) as [1,N]; par_out = its [128,NT] layout."""
                r = [psB.tile([1, 512], F32, tag=f"r{h}", name=f"r{h}") for h in range(2)]
                for h in range(2):
                    sl = slice(h * 512, (h + 1) * 512)
                    for c in range(NT):
                        nc.tensor.matmul(
                            r[h],
                            vec_par[:, c:c + 1],
                            k_tiles[c][:, sl],
                            start=(c == 0), stop=(c == NT - 1))
                    nc.vector.reciprocal(out=vec_sb[:, sl], in_=r[h])
                tp = psBt.tile([128, NT], F32, tag="tp")
                for c in range(NT):
                    nc.tensor.matmul(tp[:, c:c + 1],
                                     vec_sb[:, c * 128:(c + 1) * 128], one11,
                                     start=True, stop=True)
                nc.vector.tensor_copy(out=par_out, in_=tp)

            for _ in range(niters):
                half_iter(kt, v_par, u_sb, u_par)  # u = 1/(K~ v)
                half_iter(kn, u_par, v_sb, v_par)  # v = 1/(K~^T u)

            # emd*N = sum_ij u_i K~C_ij v_j  (extra 1/N applied at the end)
            w = [psB.tile([1, 512], F32, tag=f"r{h}", name=f"r{h}") for h in range(2)]
            dotj = work.tile([1, N], F32, tag="dotj")
            acc = work.tile([1, 1], F32, tag="acc")
            for h in range(2):
                sl = slice(h * 512, (h + 1) * 512)
                for c in range(NT):
                    nc.tensor.matmul(w[h], u_par[:, c:c + 1],
                                     kc[c][:, sl],
                                     start=(c == 0), stop=(c == NT - 1))
                nc.vector.tensor_mul(out=dotj[:, sl], in0=w[h], in1=v_sb[:, sl])
            nc.vector.reduce_sum(out=acc, in_=dotj, axis=AX.X)
            nc.vector.tensor_scalar_mul(out=out_sb[:, 5:6], in0=acc,
                                        scalar1=1.0 / float(N))

        nc.sync.dma_start(out=out[:, :], in_=out_sb)

    nc.finalize()
    return nc


def _prep_core_inputs(pc1_0, pc1_1, pc1_3, pc2, pc3):
    """Host-side sharding + feature construction. Returns in_maps list."""
    fu10_a, fv10_a = _features(pc1_0.reshape(-1, 3))
    fu2_a, fv2_a = _features(pc2.reshape(-1, 3))
    fu11_a, fv11_a = _features(pc1_1.reshape(-1, 3))
    in_maps = []
    for b in range(B):
        fu3_b, _ = _features(pc3[b])
        sl = slice(b * N, (b + 1) * N)
        sl1 = slice(b * NSEED, (b + 1) * NSEED)
        in_maps.append({
            "fu10": np.ascontiguousarray(fu10_a[:, sl]),
            "fv10b": np.ascontiguousarray(fv10_a[:, sl]),
            "fu2": np.ascontiguousarray(fu2_a[:, sl]),
            "fv2b": np.ascontiguousarray(fv2_a[:, sl]),
            "fv10a": fv10_a,
            "fv2a": fv2_a,
            "fu11": np.ascontiguousarray(fu11_a[:, sl1]),
            "fv11a": fv11_a,
            "fu3": fu3_b,
            "pc13": np.ascontiguousarray(
                pc1_3[b].reshape(N // 128, 128).T.astype(np.float32)),
        })
    return in_maps


_CACHED = {}


def kernel(pc1_0, pc1_1, pc1_3, pc2, pc3, niters=NITERS, trace=False):
    in_maps = _prep_core_inputs(pc1_0, pc1_1, pc1_3, pc2, pc3)
    key = niters
    if key not in _CACHED:
        _CACHED[key] = build_program(niters)
    nc = _CACHED[key]
    res = run_bass_kernel_spmd(nc, in_maps, list(range(B)), trace=trace)
    kernel.last_results = res

    total = np.float64(0.0)
    for b in range(B):
        q = np.asarray(res.results[b]["out"], np.float64).reshape(-1)
        total += (q[4] / (B * N)                       # confidence mse
                  + 0.5 * (q[0] + q[1]) / (B * N)      # chamfer1
                  + 0.5 * q[5]                         # emd_b
                  + q[2] / (B * NSEED) + q[3] / (B * N))  # chamfer2
    return np.float32(total)



# revision 8
# speedup vs baseline: 2.8557x; 2.8557x over previous
"""Trainium2 Bass kernel for nn_CombinedLoss_54631984005443.

Computes, over inputs pc1_0 (8,1024,3), pc1_1 (8,512,3), pc1_3 (8,1024,1),
pc2 (8,1024,3), pc3 (8,1024,3):

  loss = conf_mse + 0.5*chamfer(pc1_0, pc2) + 0.5*sum_b sinkhorn_emd(C_b)
         + chamfer(pc1_1, pc2)

Sharding: core b handles batch sample b.
 - The EMD (dominant cost: 100 Sinkhorn iterations on a 1024x1024 cost
   matrix) is per-batch -> perfectly data parallel (1 sample per core).
 - The cross-batch flattened chamfers are sharded by query rows: each core
   computes, for its 1024/512 "row" points, the min distance against ALL
   8192/4096 opposite points, in BOTH orientations, so every reduction is a
   local free-axis min. Cores emit small partial sums; the host just adds.

Key implementation choices (v2, tuned from the ntff trace of v1):
 - d^2 matrices via a K=30 bf16 "triple-split" matmul (exact to ~1e-6).
 - Sinkhorn in multiplicative form with K~ = exp(-C/eps + ln(n)):
       u = 1/(K~ v);  v = 1/(K~^T u)
   K~ kept in BF16 in both layouts (checked on host: loss-level error
   ~3e-5, vs 2e-2 budget). bf16 matmuls are single-pass on the PE
   (fp32 runs LOW/HIGH double passes) and weight loads are cheap.
 - The elementwise reciprocal runs on the SCALAR engine (AF.Reciprocal,
   ~1 elem/cycle) instead of the DVE (~6.4 cycles/elem = 3.3us per
   [1,512] row in v1 -- half the v1 runtime was DVE reciprocals).
   The reciprocal table stays loaded for the whole loop (no other
   scalar-engine op runs during phase B).
 - u/v column layouts are split into A/B half-tiles so the next matvec's
   first 4 contraction chunks only depend on the first recip + transpose,
   overlapping the second half's latency.
 - The chamfer / confidence matmuls (phase A2, 400 of them, PE+DVE only)
   are interleaved 2 per Sinkhorn half-iteration as PE filler: they absorb
   the recip/transpose latency bubbles and keep the PE busy so it holds
   its max p-state (a cold PE runs at 0.65-1.2 GHz vs 2.4 GHz ramped).
   Their scalar-engine postprocessing (sqrt/exp) is deferred to after the
   loop to avoid activation-table swaps (~1.3us each).
"""

import numpy as np
import ml_dtypes
from contextlib import ExitStack

import concourse.bass as bass
from concourse import bacc
import concourse.tile as tile
from concourse import mybir
from concourse.bass_utils import run_bass_kernel_spmd

B, N, NSEED = 8, 1024, 512
NITERS = 100
KF = 30  # feature rows for the d^2 matmul trick
F32 = mybir.dt.float32
BF16 = mybir.dt.bfloat16
AF = mybir.ActivationFunctionType
ALU = mybir.AluOpType
AX = mybir.AxisListType
LOG_N = float(np.log(N))
NT = N // 128  # 8 row tiles per 1024 points

_BF = ml_dtypes.bfloat16


def _split3(x):
    """float64 array -> three bf16 arrays h,m,l with h+m+l ~ x (to ~2^-27)."""
    h = x.astype(_BF)
    r = x - h.astype(np.float64)
    m = r.astype(_BF)
    l = (r - m.astype(np.float64)).astype(_BF)
    return h, m, l


def _features(pts):
    """pts [n,3] float -> (FU [30,n], FV [30,n]) bf16 feature matrices.

    FU(a) . FV(b) = |a|^2 + |b|^2 - 2 a.b = ||a-b||^2  (to ~1e-6 abs).
    """
    p = pts.astype(np.float64)
    n = p.shape[0]
    fu = np.zeros((KF, n), np.float64)
    fv = np.zeros((KF, n), np.float64)
    row = 0
    for c in range(3):
        h, m, l = (t.astype(np.float64) for t in _split3(p[:, c]))
        # product pairs covering (h+m+l)*(h+m+l) except l*l
        uparts = [h, h, m, m, h, l, m, l]
        vparts = [h, m, h, m, l, h, l, m]
        for uu, vv in zip(uparts, vparts):
            fu[row] = -2.0 * uu
            fv[row] = vv
            row += 1
    na = np.sum(p * p, axis=1)
    nh, nm, nl = (t.astype(np.float64) for t in _split3(na))
    for t in (nh, nm, nl):
        fu[row] = t
        fv[row] = 1.0
        row += 1
    for t in (nh, nm, nl):
        fu[row] = 1.0
        fv[row] = t
        row += 1
    assert row == KF
    return fu.astype(_BF), fv.astype(_BF)


def _P(nc, name, shape, dtype=BF16):
    return nc.declare_dram_parameter(name, list(shape), dtype, isOutput=False)


def build_program(niters=NITERS):
    nc = bacc.Bacc("TRN2")

    fu10 = _P(nc, "fu10", [KF, N])          # FU(pc1_0 batch)
    fv10b = _P(nc, "fv10b", [KF, N])        # FV(pc1_0 batch)
    fu2 = _P(nc, "fu2", [KF, N])            # FU(pc2 batch)
    fv2b = _P(nc, "fv2b", [KF, N])          # FV(pc2 batch)
    fv10a = _P(nc, "fv10a", [KF, B * N])    # FV(pc1_0 all)
    fv2a = _P(nc, "fv2a", [KF, B * N])      # FV(pc2 all)
    fu11 = _P(nc, "fu11", [KF, NSEED])      # FU(pc1_1 batch)
    fv11a = _P(nc, "fv11a", [KF, B * NSEED])  # FV(pc1_1 all)
    fu3 = _P(nc, "fu3", [KF, N])            # FU(pc3 batch)
    pc13 = _P(nc, "pc13", [128, NT], F32)
    out = nc.declare_dram_parameter("out", [1, 8], F32, isOutput=True)

    with tile.TileContext(nc) as tc, ExitStack() as top:
        sfeat = top.enter_context(tc.tile_pool(name="sfeat", bufs=1))
        bfeat = top.enter_context(tc.tile_pool(name="bfeat", bufs=1))
        kmat = top.enter_context(tc.tile_pool(name="kmat", bufs=1))
        work = top.enter_context(tc.tile_pool(name="work", bufs=1))
        consts = top.enter_context(tc.tile_pool(name="consts", bufs=1))
        mstore = top.enter_context(tc.tile_pool(name="mstore", bufs=1))

        # ---- constants
        ones_col = consts.tile([128, 1], F32, tag="ones_col")
        nc.vector.memset(ones_col, 1.0)
        one11b = consts.tile([1, 1], BF16, tag="one11b")
        nc.vector.memset(one11b, 1.0)
        lnN_vec = consts.tile([128, 1], F32, tag="lnN_vec")
        nc.vector.memset(lnN_vec, LOG_N)
        ones_row = consts.tile([1, 128], F32, tag="ones_row")
        nc.vector.memset(ones_row, 1.0)

        # ---- small (per-batch) feature tiles
        t_fu10 = sfeat.tile([KF, N], BF16, tag="fu10")
        t_fv10b = sfeat.tile([KF, N], BF16, tag="fv10b")
        t_fu2 = sfeat.tile([KF, N], BF16, tag="fu2")
        t_fv2b = sfeat.tile([KF, N], BF16, tag="fv2b")
        t_fu3 = sfeat.tile([KF, N], BF16, tag="fu3")
        t_pc13 = sfeat.tile([128, NT], F32, tag="pc13")
        for t, d in ((t_fu10, fu10), (t_fv10b, fv10b), (t_fu2, fu2),
                     (t_fv2b, fv2b), (t_fu3, fu3), (t_pc13, pc13)):
            nc.sync.dma_start(out=t, in_=d[:, :])

        # ---- big (cross-batch) feature tiles for the chamfer fills
        t_fv10a = bfeat.tile([KF, B * N], BF16, tag="fv10a")
        t_fv2a = bfeat.tile([KF, B * N], BF16, tag="fv2a")
        t_fu11 = bfeat.tile([KF, NSEED], BF16, tag="fu11")
        t_fv11a = bfeat.tile([KF, B * NSEED], BF16, tag="fv11a")
        for t0 in range(0, B * N, 2 * N):
            nc.sync.dma_start(out=t_fv10a[:, t0:t0 + 2 * N], in_=fv10a[:, t0:t0 + 2 * N])
            nc.sync.dma_start(out=t_fv2a[:, t0:t0 + 2 * N], in_=fv2a[:, t0:t0 + 2 * N])
        for t0 in range(0, B * NSEED, 2 * N):
            nc.sync.dma_start(out=t_fv11a[:, t0:t0 + 2 * N], in_=fv11a[:, t0:t0 + 2 * N])
        nc.sync.dma_start(out=t_fu11, in_=fu11[:, :])

        # ---- persistent K matrices (bf16): K~ in both layouts + K~*C
        kn = [kmat.tile([128, N], BF16, tag=f"kn{c}", name=f"kn{c}") for c in range(NT)]
        kt = [kmat.tile([128, N], BF16, tag=f"kt{c}", name=f"kt{c}") for c in range(NT)]
        kc = [kmat.tile([128, N], BF16, tag=f"kc{c}", name=f"kc{c}") for c in range(NT)]

        # ---- chamfer min-distance stores (filled during phase B)
        # quantity q: mins_q[t][:, n] = min over col-chunk n of d^2 tile t
        QSPEC = [  # (slot, fu_tile, n_row_tiles, fv_tile, n_col_chunks)
            (0, t_fu10, NT, t_fv2a, (B * N) // 512),       # chamfer1 dist2 shard
            (1, t_fu2, NT, t_fv10a, (B * N) // 512),       # chamfer1 dist1 shard
            (2, t_fu11, NSEED // 128, t_fv2a, (B * N) // 512),   # chamfer2 dist2
            (3, t_fu2, NT, t_fv11a, (B * NSEED) // 512),   # chamfer2 dist1 shard
            (4, t_fu3, NT, t_fv2b, N // 512),              # confidence gt dists
        ]
        mins = {}
        for q, _, ntile, _, nn in QSPEC:
            mins[q] = mstore.tile([128, ntile * nn], F32, tag=f"mins{q}", name=f"mins{q}")

        # ---- misc work tiles
        out_sb = work.tile([1, 8], F32, tag="out_sb")
        nc.vector.memset(out_sb, 0.0)
        u_row = work.tile([1, N], BF16, tag="u_row")
        v_row = work.tile([1, N], BF16, tag="v_row")
        u_parA = work.tile([128, NT // 2], BF16, tag="u_parA")
        u_parB = work.tile([128, NT // 2], BF16, tag="u_parB")
        v_parA = work.tile([128, NT // 2], BF16, tag="v_parA")
        v_parB = work.tile([128, NT // 2], BF16, tag="v_parB")
        scale_vec = work.tile([128, 1], F32, tag="scale_vec")  # -1/eps

        # =================================================================
        # Phase A1: EMD cost matrices C (both orientations), eps, K~, K~*C
        # =================================================================
        with tc.tile_pool(name="cmat", bufs=1) as cmat, \
             tc.tile_pool(name="psA1", bufs=4, space="PSUM") as psA1, \
             tc.tile_pool(name="wA1", bufs=2) as wA1:
            cn = [cmat.tile([128, N], F32, tag=f"cn{c}", name=f"cn{c}") for c in range(NT)]
            ct = [cmat.tile([128, N], F32, tag=f"ct{c}", name=f"ct{c}") for c in range(NT)]
            eps_acc = wA1.tile([128, 2 * NT], F32, tag="eps_acc")

            for c in range(NT):  # row tile (i for cn, j for ct)
                for h in range(2):  # 512-wide column halves
                    sl = slice(h * 512, (h + 1) * 512)
                    ps = psA1.tile([128, 512], F32, tag="d2")
                    nc.tensor.matmul(ps, t_fu10[:, c * 128:(c + 1) * 128],
                                     t_fv2b[:, sl], start=True, stop=True)
                    nc.vector.tensor_scalar_max(out=ps, in0=ps, scalar1=0.0)
                    # C = sqrt(d2); per-partition running sums for eps
                    nc.scalar.activation(out=cn[c][:, sl], in_=ps, func=AF.Sqrt,
                                         accum_out=eps_acc[:, 2 * c + h:2 * c + h + 1])
                    ps2 = psA1.tile([128, 512], F32, tag="d2")
                    nc.tensor.matmul(ps2, t_fu2[:, c * 128:(c + 1) * 128],
                                     t_fv10b[:, sl], start=True, stop=True)
                    nc.vector.tensor_scalar_max(out=ps2, in0=ps2, scalar1=0.0)
                    nc.scalar.activation(out=ct[c][:, sl], in_=ps2, func=AF.Sqrt)

            # eps = 0.02 * mean(C);  scale_vec = -1/eps broadcast to [128,1]
            s_col = wA1.tile([128, 1], F32, tag="s_col")
            nc.vector.reduce_sum(out=s_col, in_=eps_acc, axis=AX.X)
            ps_s = psA1.tile([1, 1], F32, tag="sc", bufs=1)
            nc.tensor.matmul(ps_s, s_col, ones_col, start=True, stop=True)
            s_inv = wA1.tile([1, 1], F32, tag="s_inv")
            nc.vector.reciprocal(out=s_inv, in_=ps_s)  # 1/sum(C)
            # -1/eps = -(N*N)/(0.02*sum) = s_inv * (-N*N/0.02)
            nc.vector.tensor_scalar_mul(out=s_inv, in0=s_inv,
                                        scalar1=-float(N) * float(N) / 0.02)
            ps_b = psA1.tile([128, 1], F32, tag="scb", bufs=1)
            nc.tensor.matmul(ps_b, ones_row, s_inv, start=True, stop=True)
            nc.vector.tensor_copy(out=scale_vec, in_=ps_b)

            # K~ = exp(-C/eps + ln(N)) in bf16;  K~C = K~ * C (bf16)
            # kt first: phase B's first matvec reads kt.
            for c in range(NT):
                nc.scalar.activation(out=kt[c], in_=ct[c], func=AF.Exp,
                                     bias=lnN_vec, scale=scale_vec)
            for c in range(NT):
                nc.scalar.activation(out=kn[c], in_=cn[c], func=AF.Exp,
                                     bias=lnN_vec, scale=scale_vec)
            for c in range(NT):
                nc.vector.tensor_mul(out=kc[c], in0=kn[c], in1=cn[c])

        # =================================================================
        # Phase B: Sinkhorn iterations with interleaved chamfer fills
        # =================================================================
        fills = []  # (q, t, n, fu_tile, fv_tile) work items, PE+DVE only
        for q, fu_t, ntile, fv_t, nn in QSPEC:
            for t in range(ntile):
                for n in range(nn):
                    fills.append((q, t, n, fu_t, fv_t, nn))
        fill_idx = [0]

        with tc.tile_pool(name="psB", bufs=1, space="PSUM") as psB, \
             tc.tile_pool(name="psBt", bufs=1, space="PSUM") as psBt, \
             tc.tile_pool(name="psF", bufs=2, space="PSUM") as psF:
            r0 = psB.tile([1, 512], F32, tag="r0", name="r0")
            r1 = psB.tile([1, 512], F32, tag="r1", name="r1")
            tp_u = psBt.tile([128, NT], F32, tag="tp_u", name="tp_u")
            tp_v = psBt.tile([128, NT], F32, tag="tp_v", name="tp_v")

            nc.vector.memset(v_parA, 1.0)
            nc.vector.memset(v_parB, 1.0)

            def do_fill():
                if fill_idx[0] >= len(fills):
                    return
                q, t, n, fu_t, fv_t, nn = fills[fill_idx[0]]
                fill_idx[0] += 1
                ps = psF.tile([128, 512], F32, tag="fill")
                nc.tensor.matmul(ps, fu_t[:, t * 128:(t + 1) * 128],
                                 fv_t[:, n * 512:(n + 1) * 512],
                                 start=True, stop=True)
                col = t * nn + n
                nc.vector.tensor_reduce(out=mins[q][:, col:col + 1], in_=ps,
                                        axis=AX.X, op=ALU.min)

            def half_iter(k_tiles, srcA, srcB, dst_row, dstA, dstB, tp):
                """dst = 1/(K src) in split-column layouts; dst_row gets the
                PRE-reciprocal row r = K src (bf16) for the final dot."""
                for h, r in ((0, r0), (1, r1)):
                    sl = slice(h * 512, (h + 1) * 512)
                    for c in range(NT):
                        src = srcA if c < 4 else srcB
                        nc.tensor.matmul(r, src[:, c % 4:c % 4 + 1],
                                         k_tiles[c][:, sl],
                                         start=(c == 0), stop=(c == NT - 1))
                    # PSUM row -> SBUF bf16 row on the scalar engine (Copy is
                    # in every act table -> no table swap)
                    nc.scalar.activation(out=dst_row[:, sl], in_=r,
                                         func=AF.Copy)
                # transpose row -> columns via tiny PE matmuls, half at a
                # time; elementwise reciprocal on the [128,4] columns (DVE)
                for c in range(4):
                    nc.tensor.matmul(tp[:, c:c + 1],
                                     dst_row[:, c * 128:(c + 1) * 128], one11b,
                                     start=True, stop=True)
                with nc.allow_low_precision(reason="bf16 Sinkhorn scalings"):
                    nc.vector.reciprocal(out=dstA, in_=tp[:, 0:4])
                do_fill()
                for c in range(4, NT):
                    nc.tensor.matmul(tp[:, c:c + 1],
                                     dst_row[:, c * 128:(c + 1) * 128], one11b,
                                     start=True, stop=True)
                with nc.allow_low_precision(reason="bf16 Sinkhorn scalings"):
                    nc.vector.reciprocal(out=dstB, in_=tp[:, 4:NT])
                do_fill()

            for _ in range(niters):
                half_iter(kt, v_parA, v_parB, u_row, u_parA, u_parB, tp_u)
                half_iter(kn, u_parA, u_parB, v_row, v_parA, v_parB, tp_v)

            # drain any remaining chamfer fills
            while fill_idx[0] < len(fills):
                do_fill()

            # emd*N = sum_ij u_i K~C_ij v_j = sum_j w_j / rv_j where
            # w = u^T K~C and rv = K~^T u is the pre-reciprocal row held in
            # v_row (v = 1/rv).  (extra 1/N applied at the end)
            dotj = work.tile([1, N], F32, tag="dotj")
            vrec = work.tile([1, N], F32, tag="vrec")
            acc = work.tile([1, 1], F32, tag="acc")
            nc.vector.reciprocal(out=vrec, in_=v_row)
            for h, r in ((0, r0), (1, r1)):
                sl = slice(h * 512, (h + 1) * 512)
                for c in range(NT):
                    src = u_parA if c < 4 else u_parB
                    nc.tensor.matmul(r, src[:, c % 4:c % 4 + 1],
                                     kc[c][:, sl],
                                     start=(c == 0), stop=(c == NT - 1))
                nc.vector.tensor_tensor(out=dotj[:, sl], in0=r,
                                        in1=vrec[:, sl], op=ALU.mult)
            nc.vector.reduce_sum(out=acc, in_=dotj, axis=AX.X)
            nc.vector.tensor_scalar_mul(out=out_sb[:, 5:6], in0=acc,
                                        scalar1=1.0 / float(N))

        # =================================================================
        # Phase A2 finals: reduce stored mins -> chamfer sums + confidence
        # =================================================================
        with tc.tile_pool(name="psA2", bufs=2, space="PSUM") as psA2, \
             tc.tile_pool(name="wA2", bufs=1) as wA2:
            # all sqrt-table work batched first, then exp-table work
            dmins = {}
            for q, _, ntile, _, nn in QSPEC:
                dmin = wA2.tile([128, ntile], F32, tag=f"dmin{q}", name=f"dmin{q}")
                m = mins[q]
                for t in range(ntile):
                    nc.vector.tensor_reduce(out=dmin[:, t:t + 1],
                                            in_=m[:, t * nn:(t + 1) * nn],
                                            axis=AX.X, op=ALU.min)
                nc.vector.tensor_scalar_max(out=dmin, in0=dmin, scalar1=0.0)
                nc.scalar.activation(out=dmin, in_=dmin, func=AF.Sqrt)
                dmins[q] = dmin

            for q, _, ntile, _, nn in QSPEC[:4]:
                dsum = wA2.tile([128, 1], F32, tag=f"dsum{q}", name=f"dsum{q}")
                nc.vector.reduce_sum(out=dsum, in_=dmins[q], axis=AX.X)
                ps_q = psA2.tile([1, 1], F32, tag="q", bufs=1)
                nc.tensor.matmul(ps_q, dsum, ones_col, start=True, stop=True)
                nc.vector.tensor_copy(out=out_sb[:, q:q + 1], in_=ps_q)

            # confidence: gt = exp(-min_j d(pc3_i, pc2_j)); sse vs pc1_3
            gt = wA2.tile([128, NT], F32, tag="gt")
            nc.scalar.activation(out=gt, in_=dmins[4], func=AF.Exp, scale=-1.0)
            diff = wA2.tile([128, NT], F32, tag="diff")
            nc.vector.tensor_sub(out=diff, in0=t_pc13, in1=gt)
            sse_junk = wA2.tile([128, NT], F32, tag="sse_junk")
            sse_col = wA2.tile([128, 1], F32, tag="sse_col")
            nc.vector.tensor_mul(out=sse_junk, in0=diff, in1=diff)
            nc.vector.reduce_sum(out=sse_col, in_=sse_junk, axis=AX.X)
            ps_q = psA2.tile([1, 1], F32, tag="q", bufs=1)
            nc.tensor.matmul(ps_q, sse_col, ones_col, start=True, stop=True)
            nc.vector.tensor_copy(out=out_sb[:, 4:5], in_=ps_q)

        nc.sync.dma_start(out=out[:, :], in_=out_sb)

    nc.finalize()
    return nc


def _prep_core_inputs(pc1_0, pc1_1, pc1_3, pc2, pc3):
    """Host-side sharding + feature construction. Returns in_maps list."""
    fu10_a, fv10_a = _features(pc1_0.reshape(-1, 3))
    fu2_a, fv2_a = _features(pc2.reshape(-1, 3))
    fu11_a, fv11_a = _features(pc1_1.reshape(-1, 3))
    in_maps = []
    for b in range(B):
        fu3_b, _ = _features(pc3[b])
        sl = slice(b * N, (b + 1) * N)
        sl1 = slice(b * NSEED, (b + 1) * NSEED)
        in_maps.append({
            "fu10": np.ascontiguousarray(fu10_a[:, sl]),
            "fv10b": np.ascontiguousarray(fv10_a[:, sl]),
            "fu2": np.ascontiguousarray(fu2_a[:, sl]),
            "fv2b": np.ascontiguousarray(fv2_a[:, sl]),
            "fv10a": fv10_a,
            "fv2a": fv2_a,
            "fu11": np.ascontiguousarray(fu11_a[:, sl1]),
            "fv11a": fv11_a,
            "fu3": fu3_b,
            "pc13": np.ascontiguousarray(
                pc1_3[b].reshape(N // 128, 128).T.astype(np.float32)),
        })
    return in_maps


_CACHED = {}


def kernel(pc1_0, pc1_1, pc1_3, pc2, pc3, niters=NITERS, trace=False):
    in_maps = _prep_core_inputs(pc1_0, pc1_1, pc1_3, pc2, pc3)
    key = niters
    if key not in _CACHED:
        _CACHED[key] = build_program(niters)
    nc = _CACHED[key]
    res = run_bass_kernel_spmd(nc, in_maps, list(range(B)), trace=trace)
    kernel.last_results = res

    total = np.float64(0.0)
    for b in range(B):
        q = np.asarray(res.results[b]["out"], np.float64).reshape(-1)
        total += (q[4] / (B * N)                       # confidence mse
                  + 0.5 * (q[0] + q[1]) / (B * N)      # chamfer1
                  + 0.5 * q[5]                         # emd_b
                  + q[2] / (B * NSEED) + q[3] / (B * N))  # chamfer2
    return np.float32(total)
